# revision 14
# baseline (speedup 1.0000x reference)
"""BGE-M3 scoring kernel for 8 Trainium2 NeuronCores.

Data-parallel over the 64 passages (8 per core); query side replicated.
Each core produces the [8, 8] column block of dense/sparse/colbert scores
for its passages; the host concatenates blocks along axis 1.

v3: all transposes + dtype casts moved to host-side input layout prep
(inputs arrive pre-transposed in fp8/bf16), sparse token weights move
off the PE onto DVE/Pool free-axis accumulation, norm sum-of-squares
runs as fp8 DoubleRow matmuls, and scale rows collapse into single
Rsqrt activations. PE runs only the irreducible fp8 GEMMs (projection +
token scores) plus tiny norm/broadcast passes.

Self-contained: builds the Bass program once (module cache) and runs it
via run_bass_kernel_spmd on cores 0-7.
"""
import numpy as np
import ml_dtypes
import concourse.bass as bass
import concourse.tile as tile
import concourse.mybir as mybir
from concourse.bass_utils import run_bass_kernel_spmd
from concourse.vector_clock import ScopedClock

F32 = mybir.dt.float32
F32R = mybir.dt.float32r
BF16 = mybir.dt.bfloat16
FP8 = mybir.dt.float8e4
AX = mybir.AluOpType
AF = mybir.ActivationFunctionType
X = mybir.AxisListType.X
DR = mybir.MatmulPerfMode.DoubleRow

NP8 = ml_dtypes.float8_e4m3
NP16 = ml_dtypes.bfloat16

N_CORES = 8
H = 1024
BQ, LQ = 8, 128
BP_FULL, LP = 64, 512
BP = BP_FULL // N_CORES          # 8 passages per core
HC = H // 128                    # 8 chunks of the hidden dim
TEMP = 0.02
WS = 4.0                         # fp8 scale on colbert_w
CS = 8.0                         # fp8 scale on normalized colbert vectors

# ---------------------------------------------------------------------------
# Walrus workaround: this container's neuronxcc rejects >1 sem wait per
# instruction ("Too many sync wait commands"). Split extra waits onto
# single-wait NOPs inserted just before the instruction on the same engine.
# ---------------------------------------------------------------------------
_wait_counter = [0]


def _split_multi_waits(nc):
    for fn in nc.m.functions:
        for bb in fn.blocks:
            out, changed = [], False
            for inst in bb.instructions:
                si = inst.sync_info
                if si is not None and len(si.on_wait) > 1:
                    changed = True
                    waits = list(si.on_wait)
                    for w in waits[:-1]:
                        _wait_counter[0] += 1
                        nop = mybir.InstNoOp(
                            name=f"I-waitsplit-{_wait_counter[0]}", ins=[], outs=[])
                        nop.engine = inst.engine
                        nop.sync_info = mybir.SyncInfo(on_wait=[w], on_update=[])
                        nc.register_instruction(nop)
                        out.append(nop)
                    inst.sync_info = mybir.SyncInfo(
                        on_wait=[waits[-1]], on_update=list(si.on_update))
                out.append(inst)
            if changed:
                bb.instructions = out


class _TC(tile.TileContext):
    def _drain_and_barrier(self, tick_clock, wait_clock):
        nc = self.nc
        drain_inst = nc.sync.drain()
        wait_clock.add_sem_waits(
            drain_inst.ins, ScopedClock({None: tick_clock.global_clock}))
        nc.all_engine_barrier()
        assert self.sems is not None
        popped = nc._tile_sem_poison_stack.pop()
        assert popped is self._sem_poison
        nc.clear_and_free_semaphores(list(self.sems.allocated().values()))
        nc.all_engine_barrier()

    def __exit__(self, *args):
        r = super().__exit__(*args)
        _split_multi_waits(self.nc)
        return r


def _bcast_rows(row_ap, parts=128):
    """DMA source AP replicating one DRAM row across `parts` partitions."""
    return bass.AP(tensor=row_ap.tensor, offset=row_ap.offset,
                   ap=[[0, parts]] + [list(d) for d in row_ap.ap])


# ---------------------------------------------------------------------------
# Program construction
# ---------------------------------------------------------------------------
def _build_program(repeats=1):
    nc = bass.Bass()

    d = {
        # weights / q side (pre-transposed on host)
        "wt8": nc.dram_tensor("wt8", [128, HC, H], FP8, kind="ExternalInput"),
        "cb_ws": nc.dram_tensor("cb_ws", [128, HC], F32, kind="ExternalInput"),
        "cb_h": nc.dram_tensor("cb_h", [128, HC], F32, kind="ExternalInput"),
        "sw16c": nc.dram_tensor("sw16c", [128, HC], BF16, kind="ExternalInput"),
        "sb": nc.dram_tensor("sb", [1, 1], F32, kind="ExternalInput"),
        "qhT8": nc.dram_tensor("qhT8", [128, HC, BQ * LQ], FP8,
                               kind="ExternalInput"),
        "qhT16": nc.dram_tensor("qhT16", [128, HC, BQ * LQ], BF16,
                                kind="ExternalInput"),
        "qcls16": nc.dram_tensor("qcls16", [128, HC, BQ], BF16,
                                 kind="ExternalInput"),
        "qid_cols": nc.dram_tensor("qid_cols", [128, BQ], F32,
                                   kind="ExternalInput"),
        "qmT": nc.dram_tensor("qmT", [128, BQ], F32R, kind="ExternalInput"),
        # p side (per-core shard, pre-transposed on host)
        "phT8": nc.dram_tensor("phT8", [BP, 128, HC, LP], FP8,
                               kind="ExternalInput"),
        "phT16": nc.dram_tensor("phT16", [BP, 128, HC, LP], BF16,
                                kind="ExternalInput"),
        "pcls16": nc.dram_tensor("pcls16", [128, HC, BP], BF16,
                                 kind="ExternalInput"),
        "p_ids_f": nc.dram_tensor("p_ids_f", [BP, LP], F32,
                                  kind="ExternalInput"),
        "ones_r": nc.dram_tensor("ones_r", [1, 128], F32R,
                                 kind="ExternalInput"),
        "ones_c": nc.dram_tensor("ones_c", [128, 1], F32R,
                                 kind="ExternalInput"),
    }
    o = {
        "dense": nc.dram_tensor("dense", [BQ, BP], F32, kind="ExternalOutput"),
        "sparse": nc.dram_tensor("sparse", [BQ, BP], F32,
                                 kind="ExternalOutput"),
        "colbert": nc.dram_tensor("colbert", [BQ, BP], F32,
                                  kind="ExternalOutput"),
    }

    with _TC(nc) as tc:
        for _ in range(repeats):
            _emit(nc, tc, d, o)
    return nc


def _emit(nc, tc, d, o):
    from contextlib import ExitStack
    es = ExitStack()
    with es:
        es.enter_context(nc.allow_low_precision(reason="fp8/bf16 is the target precision"))
        persist = es.enter_context(tc.tile_pool(name="persist", bufs=1))
        dram = es.enter_context(tc.tile_pool(name="dram", bufs=1, space="DRAM"))
        ps_mm = es.enter_context(tc.tile_pool(name="ps_mm", bufs=3, space="PSUM"))
        ps_row = es.enter_context(tc.tile_pool(name="ps_row", bufs=2, space="PSUM"))
        ps_sb = es.enter_context(tc.tile_pool(name="ps_sb", bufs=2, space="PSUM"))

        # ---- persistent small tiles --------------------------------------
        ones_c16 = persist.tile([128, 1], BF16, tag="ones_c16")
        nc.vector.memset(ones_c16[:], 1.0)
        ones_r = persist.tile([1, 128], F32R, tag="ones_r")
        nc.sync.dma_start(out=ones_r[:], in_=d["ones_r"][:])
        ones_cq = persist.tile([128, 1], F32R, tag="ones_cq")
        nc.sync.dma_start(out=ones_cq[:], in_=d["ones_c"][:])
        # fp8 ones for DoubleRow norm sums (width 16 keeps the k-pair byte
        # stride a multiple of 16)
        ones8 = persist.tile([128, HC, 16], FP8, tag="ones8")
        for k in range(HC):
            nc.vector.memset(ones8[:, k, :], 1.0)
        sw16c = persist.tile([128, HC], BF16, tag="sw16c")
        nc.sync.dma_start(out=sw16c[:], in_=d["sw16c"][:])
        sb_sb = persist.tile([1, 1], F32, tag="sb")
        nc.sync.dma_start(out=sb_sb[:], in_=d["sb"][:])
        eps_sb = persist.tile([1, 1], F32, tag="eps")
        nc.vector.memset(eps_sb[:], 1e-24)
        cb_ws = persist.tile([128, HC], F32, tag="cb")
        nc.sync.dma_start(out=cb_ws[:], in_=d["cb_ws"][:])
        cb_h = persist.tile([128, HC], F32, tag="cbh")
        nc.sync.dma_start(out=cb_h[:], in_=d["cb_h"][:])
        qid_cols = persist.tile([128, BQ], F32, tag="qid_cols")
        nc.sync.dma_start(out=qid_cols[:], in_=d["qid_cols"][:])
        qmT = persist.tile([128, BQ], F32R, tag="qmT")
        nc.sync.dma_start(out=qmT[:], in_=d["qmT"][:])
        qcls16 = persist.tile([128, HC, BQ], BF16, tag="qcls16")
        nc.sync.dma_start(out=qcls16[:], in_=d["qcls16"][:])
        pcls16 = persist.tile([128, HC, BP], BF16, tag="pcls16")
        nc.sync.dma_start(out=pcls16[:], in_=d["pcls16"][:])

        wt8 = persist.tile([128, HC, H], FP8, tag="wt8")
        nc.sync.dma_start(out=wt8[:], in_=d["wt8"][:])
        qhT8 = persist.tile([128, HC, BQ * LQ], FP8, tag="qhT8")
        nc.sync.dma_start(out=qhT8[:], in_=d["qhT8"][:])
        qcolT8 = persist.tile([128, HC, BQ * LQ], FP8, tag="qcolT8")

        rmax = [persist.tile([128, BP], F32R, tag=f"rmax{i}", name=f"rmax{i}")
                for i in range(BQ)]
        ssum = [persist.tile([128, BP], F32R, tag=f"ssum{i}", name=f"ssum{i}")
                for i in range(BQ)]
        qw = [persist.tile([128, 1], F32R, tag=f"qw{i}", name=f"qw{i}")
              for i in range(BQ)]
        qivp = [persist.tile([128, 2], F32R, tag=f"qivp{t}", name=f"qivp{t}")
                for t in range(BQ // 2)]

        d_twp = dram.tile([BP, LP], F32R, name="d_twp")
        d_twq = dram.tile([1, BQ * LQ], F32R, name="d_twq")
        d_rq = dram.tile([1, BQ], F32, name="d_rq")

        # per-passage pools
        phT8_p = es.enter_context(tc.tile_pool(name="phT8", bufs=2))
        phT16_p = es.enter_context(tc.tile_pool(name="phT16", bufs=2))
        brow_p = es.enter_context(tc.tile_pool(name="brow", bufs=3))
        vraw_p = es.enter_context(tc.tile_pool(name="vraw", bufs=2))
        vsq_p = es.enter_context(tc.tile_pool(name="vsq", bufs=2))
        pcol_p = es.enter_context(tc.tile_pool(name="pcol", bufs=3))
        sb16_p = es.enter_context(tc.tile_pool(name="sb16", bufs=2))
        mt_p = es.enter_context(tc.tile_pool(name="mt", bufs=2))
        twp_p = es.enter_context(tc.tile_pool(name="twp", bufs=3))
        row_p = es.enter_context(tc.tile_pool(name="rowp", bufs=2))

        def load(j):
            phT8_t = phT8_p.tile([128, HC, LP], FP8, tag="phT8")
            nc.sync.dma_start(out=phT8_t[:], in_=d["phT8"][j])
            phT16_t = phT16_p.tile([128, HC, LP], BF16, tag="phT16")
            nc.sync.dma_start(out=phT16_t[:], in_=d["phT16"][j])
            pidB = brow_p.tile([128, LP], F32, tag="pidB")
            nc.gpsimd.dma_start(out=pidB[:], in_=_bcast_rows(d["p_ids_f"][j:j + 1, :]))
            return phT8_t, phT16_t, pidB

        loads = {0: load(0)}

        # ================= q side =========================================
        with tc.tile_pool(name="qtmp", bufs=1) as qtmp, \
             tc.tile_pool(name="qvraw", bufs=2) as qvraw_p, \
             tc.tile_pool(name="qvsq", bufs=2) as qvsq_p:
            qhT16 = qtmp.tile([128, HC, BQ * LQ], BF16, tag="qhT16")
            nc.sync.dma_start(out=qhT16[:], in_=d["qhT16"][:])

            # sparse token weights for q: PE matvec over hidden chunks,
            # relu row, DRAM bounce to per-token columns
            twq_row = qtmp.tile([1, BQ * LQ], F32R, tag="twq_row")
            for g in range(2):
                ptw = ps_row.tile([1, 512], F32, tag="row")
                for k in range(HC):
                    nc.tensor.matmul(ptw[:], sw16c[:, k:k + 1],
                                     qhT16[:, k, g * 512:(g + 1) * 512],
                                     start=(k == 0), stop=(k == HC - 1))
                nc.scalar.activation(out=twq_row[:, g * 512:(g + 1) * 512],
                                     in_=ptw[:], func=AF.Relu, bias=sb_sb[:],
                                     scale=1.0)
            nc.gpsimd.dma_start(out=d_twq[:], in_=twq_row[:])
            twq_cols = qtmp.tile([128, BQ], F32R, tag="twq_cols")
            nc.sync.dma_start(
                out=twq_cols[:],
                in_=bass.AP(tensor=d_twq.tensor, offset=0,
                            ap=[[1, 128], [128, BQ]]))
            for i in range(BQ):
                nc.vector.tensor_scalar(
                    out=qw[i][:], in0=twq_cols[:, i:i + 1], scalar1=1.0,
                    scalar2=None, op0=AX.mult)

            # colbert projection + normalize (2 column groups of 512)
            for g in range(2):
                cols = slice(g * 512, (g + 1) * 512)
                vraw_q = qvraw_p.tile([128, HC, 512], BF16, tag="qvraw")
                vsq8_q = qvsq_p.tile([128, HC, 512], FP8, tag="qvsq")
                for m in range(HC):
                    pmm = ps_mm.tile([128, 512], F32, tag="mm")
                    for t in range(HC // 2):
                        nc.tensor.matmul(
                            pmm[:],
                            wt8[:, 2 * t:2 * t + 2, m * 128:(m + 1) * 128],
                            qhT8[:, 2 * t:2 * t + 2, cols],
                            start=(t == 0), stop=(t == HC // 2 - 1),
                            perf_mode=DR)
                    if m % 2 == 0:
                        nc.scalar.activation(out=vsq8_q[:, m, :], in_=pmm[:],
                                             func=AF.Square,
                                             bias=cb_h[:, m:m + 1], scale=0.5)
                    nc.scalar.activation(out=vraw_q[:, m, :], in_=pmm[:],
                                         func=AF.Identity,
                                         bias=cb_ws[:, m:m + 1], scale=1.0)
                    if m % 2 == 1:
                        nc.vector.scalar_tensor_tensor(
                            out=vsq8_q[:, m, :], in0=vraw_q[:, m, :],
                            scalar=0.25, in1=vraw_q[:, m, :],
                            op0=AX.mult, op1=AX.mult)
                pss = ps_row.tile([1, 512], F32, tag="row")
                for t in range(HC // 2):
                    nc.tensor.matmul(
                        pss[:], ones8[:, 2 * t:2 * t + 2, 0:1],
                        vsq8_q[:, 2 * t:2 * t + 2, :],
                        start=(t == 0), stop=(t == HC // 2 - 1), perf_mode=DR)
                nrow = qtmp.tile([1, 512], F32, tag=f"qnrow{g}", name=f"qnrow{g}")
                nc.scalar.activation(out=nrow[:], in_=pss[:], func=AF.Sqrt,
                                     bias=eps_sb[:], scale=4.0 / (CS * CS))
                srow = qtmp.tile([1, 512], F32R, tag=f"qsrow{g}", name=f"qsrow{g}")
                nc.vector.reciprocal(out=srow[:], in_=nrow[:])
                sB = ps_sb.tile([128, 512], F32, tag="sb")
                nc.tensor.matmul(sB[:], ones_r[:], srow[:], start=True, stop=True)
                sB16 = sb16_p.tile([128, 512], BF16, tag="sb16")
                nc.scalar.copy(out=sB16[:], in_=sB[:])
                for m in range(HC):
                    if m < 4:
                        nc.vector.tensor_mul(qcolT8[:, m, cols],
                                             vraw_q[:, m, :], sB[:])
                    else:
                        nc.gpsimd.tensor_mul(qcolT8[:, m, cols],
                                             vraw_q[:, m, :], sB16[:])

            # qlen -> 1/(qlen*TEMP*CS*CS) broadcast pair columns
            pql = ps_row.tile([1, BQ], F32, tag="row")
            nc.tensor.matmul(pql[:], ones_cq[0:127, :], qmT[0:127, :],
                             start=True, stop=True)
            qiv_row = qtmp.tile([1, BQ], F32R, tag="qiv")
            nc.vector.tensor_scalar(out=qiv_row[:], in0=pql[:],
                                    scalar1=TEMP * CS * CS,
                                    scalar2=None, op0=AX.mult)
            nc.vector.reciprocal(out=qiv_row[:], in_=qiv_row[:])
            for t in range(BQ // 2):
                pqc = ps_row.tile([128, 2], F32, tag="row")
                nc.tensor.matmul(pqc[:], ones_r[:], qiv_row[:, 2 * t:2 * t + 2],
                                 start=True, stop=True)
                nc.scalar.copy(out=qivp[t][:], in_=pqc[:])

        # ================= passage loop ===================================
        # steady-state PE order per iteration j:
        #   proj(j) x32, psc(j-1) q0..6, norm(j) x4, psc(j-1) q7, sB(j)
        state = {}

        def proj_tw(j):
            phT8_t, phT16_t, pidB = loads.pop(j)
            vraw = vraw_p.tile([128, HC, LP], BF16, tag="vraw")
            vsq8 = vsq_p.tile([128, HC, LP], FP8, tag="vsq")
            for m in range(HC):
                pmm = ps_mm.tile([128, LP], F32, tag="mm")
                for t in range(HC // 2):
                    nc.tensor.matmul(
                        pmm[:],
                        wt8[:, 2 * t:2 * t + 2, m * 128:(m + 1) * 128],
                        phT8_t[:, 2 * t:2 * t + 2, :],
                        start=(t == 0), stop=(t == HC // 2 - 1), perf_mode=DR)
                # Square first: the norm matmul is the tighter same-iter dep
                nc.scalar.activation(out=vsq8[:, m, :], in_=pmm[:],
                                     func=AF.Square,
                                     bias=cb_h[:, m:m + 1], scale=0.5)
                nc.scalar.activation(out=vraw[:, m, :], in_=pmm[:],
                                     func=AF.Identity,
                                     bias=cb_ws[:, m:m + 1], scale=1.0)
            # sparse token weights: PE matvec, relu row, bounce + broadcast
            ptw = ps_row.tile([1, LP], F32, tag="row")
            for k in range(HC):
                nc.tensor.matmul(ptw[:], sw16c[:, k:k + 1], phT16_t[:, k, :],
                                 start=(k == 0), stop=(k == HC - 1))
            twp_row = twp_p.tile([1, LP], F32R, tag="twp_row")
            nc.scalar.activation(out=twp_row[:], in_=ptw[:], func=AF.Relu,
                                 bias=sb_sb[:], scale=1.0)
            nc.gpsimd.dma_start(out=d_twp[j:j + 1, :], in_=twp_row[:])
            twpB = brow_p.tile([128, LP], F32R, tag="twpB")
            nc.gpsimd.dma_start(out=twpB[:], in_=_bcast_rows(d_twp[j:j + 1, :]))
            state[j] = [vraw, vsq8, twpB, pidB, None]

        def norm(j):
            vsq8 = state[j][1]
            pss = ps_row.tile([1, LP], F32, tag="row")
            for t in range(HC // 2):
                nc.tensor.matmul(
                    pss[:], ones8[:, 2 * t:2 * t + 2, 0:1],
                    vsq8[:, 2 * t:2 * t + 2, :],
                    start=(t == 0), stop=(t == HC // 2 - 1), perf_mode=DR)
            return pss

        def scale(j, pss):
            vraw = state[j][0]
            nrow = row_p.tile([1, LP], F32, tag="nrow")
            nc.scalar.activation(out=nrow[:], in_=pss[:], func=AF.Sqrt,
                                 bias=eps_sb[:], scale=4.0 / (CS * CS))
            srow = row_p.tile([1, LP], F32R, tag="srow")
            nc.vector.reciprocal(out=srow[:], in_=nrow[:])
            sB = ps_sb.tile([128, LP], F32, tag="sb")
            nc.tensor.matmul(sB[:], ones_r[:], srow[:], start=True, stop=True)
            sB16 = sb16_p.tile([128, LP], BF16, tag="sb16")
            nc.scalar.copy(out=sB16[:], in_=sB[:])
            pcolT8 = pcol_p.tile([128, HC, LP], FP8, tag="pct8")
            for m in range(HC):
                if m < 4:
                    nc.vector.tensor_mul(pcolT8[:, m, :], vraw[:, m, :], sB[:])
                else:
                    nc.gpsimd.tensor_mul(pcolT8[:, m, :], vraw[:, m, :],
                                         sB16[:])
            state[j][4] = pcolT8

        def scores(j, qs):
            _, _, twpB, pidB, pcolT8 = state[j]
            for i in qs:
                psc = ps_mm.tile([127, LP], F32, tag="mm")
                for t in range(HC // 2):
                    nc.tensor.matmul(
                        psc[:],
                        qcolT8[:, 2 * t:2 * t + 2, i * 128 + 1:(i + 1) * 128],
                        pcolT8[:, 2 * t:2 * t + 2, :],
                        start=(t == 0), stop=(t == HC // 2 - 1), perf_mode=DR)
                mt = mt_p.tile([128, LP], F32, tag="mt")
                nc.vector.scalar_tensor_tensor(
                    out=mt[:], in0=pidB[:], scalar=qid_cols[:, i:i + 1],
                    in1=twpB[:], op0=AX.is_equal, op1=AX.mult,
                    accum_out=ssum[i][:, j:j + 1])
                nc.vector.reduce_max(out=rmax[i][0:127, j:j + 1],
                                     in_=psc[:, 1:LP], axis=X)

        for j in range(BP):
            if j + 1 < BP:
                loads[j + 1] = load(j + 1)
            proj_tw(j)
            if j > 0:
                scores(j - 1, range(0, 7))
            pss = norm(j)
            if j > 0:
                scores(j - 1, range(7, 8))
                del state[j - 1]
            scale(j, pss)
        scores(BP - 1, range(BQ))
        del state[BP - 1]

        # ================= finals =========================================
        with tc.tile_pool(name="fin", bufs=1) as fin:
            cst_all = fin.tile([1, BQ, BP], F32, tag="cst_all")
            sst_all = fin.tile([1, BQ, BP], F32, tag="sst_all")
            for i in range(BQ):
                pcbi = ps_row.tile([1, BP], F32, tag="row")
                nc.tensor.matmul(pcbi[:], qivp[i // 2][0:127, i % 2:i % 2 + 1],
                                 rmax[i][0:127, :], start=True, stop=True)
                nc.scalar.copy(out=cst_all[:, i, :], in_=pcbi[:])
                pspi = ps_row.tile([1, BP], F32, tag="row")
                nc.tensor.matmul(pspi[:], qw[i][:], ssum[i][:],
                                 start=True, stop=True)
                nc.scalar.activation(out=sst_all[:, i, :], in_=pspi[:],
                                     func=AF.Copy, scale=1.0 / TEMP)
            nc.sync.dma_start(
                out=bass.AP(tensor=o["colbert"].ap().tensor, offset=0,
                            ap=[[0, 1], [1, BQ * BP]]),
                in_=cst_all[:].rearrange("p i j -> p (i j)"))
            nc.sync.dma_start(
                out=bass.AP(tensor=o["sparse"].ap().tensor, offset=0,
                            ap=[[0, 1], [1, BQ * BP]]),
                in_=sst_all[:].rearrange("p i j -> p (i j)"))

            # dense scores (bf16 cls tiles)
            pd = ps_sb.tile([BQ, BP], F32, tag="sb")
            pqn = ps_row.tile([1, BQ], F32, tag="row")
            ppn = ps_row.tile([1, BP], F32, tag="row")
            for k in range(HC):
                nc.tensor.matmul(pd[:], qcls16[:, k, :], pcls16[:, k, :],
                                 start=(k == 0), stop=(k == HC - 1))
                qsq = fin.tile([128, BQ], BF16, tag="qsq")
                nc.scalar.activation(out=qsq[:], in_=qcls16[:, k, :],
                                     func=AF.Square)
                nc.tensor.matmul(pqn[:], ones_c16[:], qsq[:],
                                 start=(k == 0), stop=(k == HC - 1))
                psq = fin.tile([128, BP], BF16, tag="psq")
                nc.scalar.activation(out=psq[:], in_=pcls16[:, k, :],
                                     func=AF.Square)
                nc.tensor.matmul(ppn[:], ones_c16[:], psq[:],
                                 start=(k == 0), stop=(k == HC - 1))
            pdsb = fin.tile([BQ, BP], F32, tag="pdsb")
            nc.scalar.copy(out=pdsb[:], in_=pd[:])
            rq_row = fin.tile([1, BQ], F32, tag="rq_row")
            nc.scalar.activation(out=rq_row[:], in_=pqn[:], func=AF.Sqrt,
                                 bias=eps_sb[:])
            nc.vector.reciprocal(out=rq_row[:], in_=rq_row[:])
            rp_row = fin.tile([1, BP], F32R, tag="rp_row")
            nc.scalar.activation(out=rp_row[:], in_=ppn[:], func=AF.Sqrt,
                                 bias=eps_sb[:])
            nc.vector.reciprocal(out=rp_row[:], in_=rp_row[:])
            # rq as a column via DRAM bounce
            nc.sync.dma_start(out=d_rq[:], in_=rq_row[:])
            rq_col = fin.tile([BQ, 1], F32, tag="rq_col")
            nc.sync.dma_start(
                out=rq_col[:],
                in_=bass.AP(tensor=d_rq.tensor, offset=0, ap=[[1, BQ], [0, 1]]))
            prpb = ps_row.tile([BQ, BP], F32, tag="row")
            nc.tensor.matmul(prpb[:], ones_r[:, 0:BQ], rp_row[:],
                             start=True, stop=True)
            dmul = fin.tile([BQ, BP], F32, tag="dmul")
            nc.vector.tensor_mul(dmul[:], pdsb[:], prpb[:])
            dout = fin.tile([BQ, BP], F32, tag="dout")
            nc.vector.tensor_scalar(out=dout[:], in0=dmul[:], scalar1=rq_col[:],
                                    scalar2=1.0 / TEMP, op0=AX.mult, op1=AX.mult)
            nc.sync.dma_start(out=o["dense"][:], in_=dout[:])


# ---------------------------------------------------------------------------
# Host-side driver
# ---------------------------------------------------------------------------
_PROGRAM = None


def _get_program():
    global _PROGRAM
    if _PROGRAM is None:
        _PROGRAM = _build_program()
    return _PROGRAM


def _prep_ids(ids, sentinel):
    f = ids.astype(np.float32)
    return np.where(ids <= 3, np.float32(sentinel), f).astype(np.float32)


def make_in_maps(q_hidden, p_hidden, q_mask, p_mask, q_ids, p_ids,
                 colbert_w, colbert_b, sparse_w, sparse_b):
    q_hidden = np.asarray(q_hidden, np.float32)
    p_hidden = np.asarray(p_hidden, np.float32)
    q_mask = np.asarray(q_mask, np.float32)
    colbert_w = np.asarray(colbert_w, np.float32)
    colbert_b = np.asarray(colbert_b, np.float32)
    sparse_w = np.asarray(sparse_w, np.float32)
    sparse_b = np.asarray(sparse_b, np.float32).reshape(1, 1)
    q_ids = np.asarray(q_ids)
    p_ids = np.asarray(p_ids)

    C = np.ascontiguousarray
    # weights: wt8[p, k, d] = WS * W[d, k*128+p]
    wt8 = C((colbert_w.T * WS).reshape(HC, 128, H).transpose(1, 0, 2)).astype(NP8)
    cb_ws = C((colbert_b * WS).reshape(HC, 128).T)
    cb_h = C(cb_ws * 0.5)
    sw16c = C(sparse_w.reshape(HC, 128).T).astype(NP16)
    # q side: qhT8[p, k, i*128+l] = q_hidden[i, l, k*128+p]
    qhT8 = C(q_hidden.reshape(BQ, LQ, HC, 128).transpose(3, 2, 0, 1)
             .reshape(128, HC, BQ * LQ)).astype(NP8)
    qhT16 = C(q_hidden.reshape(BQ, LQ, HC, 128).transpose(3, 2, 0, 1)
              .reshape(128, HC, BQ * LQ)).astype(NP16)
    qcls16 = C(q_hidden[:, 0, :].T.reshape(HC, 128, BQ)
               .transpose(1, 0, 2)).astype(NP16)
    qid_cols = C(_prep_ids(q_ids, -2.0).T)
    qmT = np.zeros((128, BQ), np.float32)
    qmT[0:127, :] = q_mask[:, 1:128].T

    in_maps = []
    for c in range(N_CORES):
        sl = slice(c * BP, (c + 1) * BP)
        ph = p_hidden[sl]
        phT = ph.reshape(BP, LP, HC, 128).transpose(0, 3, 2, 1)
        phT8 = C(phT).astype(NP8)
        phT16 = C(phT).astype(NP16)
        pcls16 = C(ph[:, 0, :].T.reshape(HC, 128, BP)
                   .transpose(1, 0, 2)).astype(NP16)
        in_maps.append({
            "wt8": wt8,
            "cb_ws": cb_ws,
            "cb_h": cb_h,
            "sw16c": sw16c,
            "sb": sparse_b,
            "qhT8": qhT8,
            "qhT16": qhT16,
            "qcls16": qcls16,
            "qid_cols": qid_cols,
            "qmT": qmT,
            "phT8": phT8,
            "phT16": phT16,
            "pcls16": pcls16,
            "p_ids_f": C(_prep_ids(p_ids[sl], -1.0)),
            "ones_r": np.ones((1, 128), np.float32),
            "ones_c": np.ones((128, 1), np.float32),
        })
    return in_maps


def kernel(q_hidden, p_hidden, q_mask, p_mask, q_ids, p_ids,
           colbert_w, colbert_b, sparse_w, sparse_b):
    nc = _get_program()
    in_maps = make_in_maps(q_hidden, p_hidden, q_mask, p_mask, q_ids, p_ids,
                           colbert_w, colbert_b, sparse_w, sparse_b)
    res = run_bass_kernel_spmd(nc, in_maps, list(range(N_CORES)))
    dense = np.concatenate([res.results[c]["dense"] for c in range(N_CORES)], axis=1)
    sparse = np.concatenate([res.results[c]["sparse"] for c in range(N_CORES)], axis=1)
    colbert = np.concatenate([res.results[c]["colbert"] for c in range(N_CORES)],
                             axis=1)
    return dense, sparse, colbert


# revision 31
# speedup vs baseline: 1.2815x; 1.2815x over previous
"""BGE-M3 scoring kernel for 8 Trainium2 NeuronCores.

Data-parallel over the 64 passages (8 per core); query side replicated.
Each core produces the [8, 8] column block of dense/sparse/colbert scores
for its passages; the host concatenates blocks along axis 1.

v3: all transposes + dtype casts moved to host-side input layout prep
(inputs arrive pre-transposed in fp8/bf16), sparse token weights move
off the PE onto DVE/Pool free-axis accumulation, norm sum-of-squares
runs as fp8 DoubleRow matmuls, and scale rows collapse into single
Rsqrt activations. PE runs only the irreducible fp8 GEMMs (projection +
token scores) plus tiny norm/broadcast passes.

Self-contained: builds the Bass program once (module cache) and runs it
via run_bass_kernel_spmd on cores 0-7.
"""
import numpy as np
import ml_dtypes
import concourse.bass as bass
import concourse.tile as tile
import concourse.mybir as mybir
from concourse.bass_utils import run_bass_kernel_spmd
from concourse.vector_clock import ScopedClock

F32 = mybir.dt.float32
F32R = mybir.dt.float32r
BF16 = mybir.dt.bfloat16
FP8 = mybir.dt.float8e4
AX = mybir.AluOpType
AF = mybir.ActivationFunctionType
X = mybir.AxisListType.X
DR = mybir.MatmulPerfMode.DoubleRow

NP8 = ml_dtypes.float8_e4m3
NP16 = ml_dtypes.bfloat16

N_CORES = 8
H = 1024
BQ, LQ = 8, 128
BP_FULL, LP = 64, 512
BP = BP_FULL // N_CORES          # 8 passages per core
HC = H // 128                    # 8 chunks of the hidden dim
TEMP = 0.02
WS = 4.0                         # fp8 scale on colbert_w
CS = 8.0                         # fp8 scale on normalized colbert vectors

# ---------------------------------------------------------------------------
# Walrus workaround: this container's neuronxcc rejects >1 sem wait per
# instruction ("Too many sync wait commands"). Split extra waits onto
# single-wait NOPs inserted just before the instruction on the same engine.
# ---------------------------------------------------------------------------
_wait_counter = [0]


def _split_multi_waits(nc):
    for fn in nc.m.functions:
        for bb in fn.blocks:
            out, changed = [], False
            for inst in bb.instructions:
                si = inst.sync_info
                if si is not None and len(si.on_wait) > 1:
                    changed = True
                    waits = list(si.on_wait)
                    for w in waits[:-1]:
                        _wait_counter[0] += 1
                        nop = mybir.InstNoOp(
                            name=f"I-waitsplit-{_wait_counter[0]}", ins=[], outs=[])
                        nop.engine = inst.engine
                        nop.sync_info = mybir.SyncInfo(on_wait=[w], on_update=[])
                        nc.register_instruction(nop)
                        out.append(nop)
                    inst.sync_info = mybir.SyncInfo(
                        on_wait=[waits[-1]], on_update=list(si.on_update))
                out.append(inst)
            if changed:
                bb.instructions = out


class _TC(tile.TileContext):
    def _drain_and_barrier(self, tick_clock, wait_clock):
        nc = self.nc
        drain_inst = nc.sync.drain()
        wait_clock.add_sem_waits(
            drain_inst.ins, ScopedClock({None: tick_clock.global_clock}))
        nc.all_engine_barrier()
        assert self.sems is not None
        popped = nc._tile_sem_poison_stack.pop()
        assert popped is self._sem_poison
        nc.clear_and_free_semaphores(list(self.sems.allocated().values()))
        nc.all_engine_barrier()

    def __exit__(self, *args):
        r = super().__exit__(*args)
        _split_multi_waits(self.nc)
        return r


def _bcast_rows(row_ap, parts=128):
    """DMA source AP replicating one DRAM row across `parts` partitions."""
    return bass.AP(tensor=row_ap.tensor, offset=row_ap.offset,
                   ap=[[0, parts]] + [list(d) for d in row_ap.ap])


# ---------------------------------------------------------------------------
# Program construction
# ---------------------------------------------------------------------------
def _build_program(repeats=1):
    nc = bass.Bass()

    d = {
        # weights / q side (pre-transposed on host)
        "wt8": nc.dram_tensor("wt8", [128, HC, H], FP8, kind="ExternalInput"),
        "packf": nc.dram_tensor("packf", [128, 33], F32, kind="ExternalInput"),
        "qmT": nc.dram_tensor("qmT", [128, BQ], F32R, kind="ExternalInput"),
        "pack16": nc.dram_tensor("pack16", [128, HC, 17], BF16,
                                 kind="ExternalInput"),
        "qhT8": nc.dram_tensor("qhT8", [128, HC, BQ * LQ], FP8,
                               kind="ExternalInput"),
        "qhT16": nc.dram_tensor("qhT16", [128, HC, BQ * LQ], BF16,
                                kind="ExternalInput"),
        "qcls16": nc.dram_tensor("qcls16", [128, HC, BQ], BF16,
                                 kind="ExternalInput"),
        "qid_cols": nc.dram_tensor("qid_cols", [128, BQ], F32,
                                   kind="ExternalInput"),
        "qmT": nc.dram_tensor("qmT", [128, BQ], F32R, kind="ExternalInput"),
        # p side (per-core shard, pre-transposed on host)
        "phT8": nc.dram_tensor("phT8", [BP, 128, HC, LP], FP8,
                               kind="ExternalInput"),
        "phT16": nc.dram_tensor("phT16", [BP, 128, HC, LP], BF16,
                                kind="ExternalInput"),
        "pcls16": nc.dram_tensor("pcls16", [128, HC, BP], BF16,
                                 kind="ExternalInput"),
        "phot16": nc.dram_tensor("phot16", [BP, 128, 4, 1024], BF16,
                                 kind="ExternalInput"),
        "qhot16": nc.dram_tensor("qhot16", [128, 8, BQ, 128], BF16,
                                 kind="ExternalInput"),
        "ones_r": nc.dram_tensor("ones_r", [1, 128], F32R,
                                 kind="ExternalInput"),
        "ones_c": nc.dram_tensor("ones_c", [128, 1], F32R,
                                 kind="ExternalInput"),
    }
    o = {
        "dense": nc.dram_tensor("dense", [BQ, BP], F32, kind="ExternalOutput"),
        "sparse": nc.dram_tensor("sparse", [BQ, BP], F32,
                                 kind="ExternalOutput"),
        "colbert": nc.dram_tensor("colbert", [BQ, BP], F32,
                                  kind="ExternalOutput"),
    }

    with _TC(nc) as tc:
        for _ in range(repeats):
            _emit(nc, tc, d, o)
    return nc


def _emit(nc, tc, d, o):
    from contextlib import ExitStack
    es = ExitStack()
    with es:
        es.enter_context(nc.allow_low_precision(reason="fp8/bf16 is the target precision"))
        persist = es.enter_context(tc.tile_pool(name="persist", bufs=1))
        dram = es.enter_context(tc.tile_pool(name="dram", bufs=1, space="DRAM"))
        ps_mm = es.enter_context(tc.tile_pool(name="ps_mm", bufs=5, space="PSUM"))
        ps_row = es.enter_context(tc.tile_pool(name="ps_row", bufs=2, space="PSUM"))
        ps_sb = es.enter_context(tc.tile_pool(name="ps_sb", bufs=1, space="PSUM"))

        # ---- persistent small tiles --------------------------------------
        ones_c16 = persist.tile([128, 1], BF16, tag="ones_c16")
        nc.vector.memset(ones_c16[:], 1.0)
        ones_r = persist.tile([1, 128], F32R, tag="ones_r")
        nc.scalar.dma_start(out=ones_r[:], in_=d["ones_r"][:])
        ones_cq = persist.tile([128, 1], F32R, tag="ones_cq")
        nc.scalar.dma_start(out=ones_cq[:], in_=d["ones_c"][:])
        # fp8 ones for DoubleRow norm sums (width 16 keeps the k-pair byte
        # stride a multiple of 16)
        ones8 = persist.tile([128, HC, 16], FP8, tag="ones8")
        for k in range(HC):
            nc.vector.memset(ones8[:, k, :], 1.0)
        packf = persist.tile([128, 33], F32, tag="packf")
        nc.scalar.dma_start(out=packf[:], in_=d["packf"][:])
        pack16 = persist.tile([128, HC, 17], BF16, tag="pack16")
        nc.scalar.dma_start(out=pack16[:], in_=d["pack16"][:])
        qmT = persist.tile([128, BQ], F32R, tag="qmT")
        nc.scalar.dma_start(out=qmT[:], in_=d["qmT"][:])
        eps_sb = persist.tile([1, 1], F32, tag="eps")
        nc.vector.memset(eps_sb[:], 1e-24)
        wt8 = persist.tile([128, HC, H], FP8, tag="wt8")
        nc.sync.dma_start(out=wt8[:], in_=d["wt8"][:])
        qhT8 = persist.tile([128, HC, BQ * LQ], FP8, tag="qhT8")
        nc.sync.dma_start(out=qhT8[:], in_=d["qhT8"][:])
        qcolT8 = persist.tile([128, HC, BQ * LQ], FP8, tag="qcolT8")

        rmax_all = persist.tile([128, BQ * BP], F32R, tag="rmax_all")
        qhot16 = persist.tile([128, 8, BQ, 128], BF16, tag="qhot16")
        nc.sync.dma_start(out=qhot16[:], in_=d["qhot16"][:])
        S_sb = persist.tile([128, HC, BP], BF16, tag="S_sb")
        qwB = [persist.tile([BQ, 128], F32R, tag=f"qwB{i}", name=f"qwB{i}")
               for i in range(BQ)]
        qiv_row = persist.tile([1, BQ], F32, tag="qiv")

        d_twp = dram.tile([BP, LP], BF16, name="d_twp")
        d_twq = dram.tile([1, BQ * LQ], F32R, name="d_twq")
        d_rq = dram.tile([1, BQ], F32, name="d_rq")

        # per-passage pools
        phT8_p = es.enter_context(tc.tile_pool(name="phT8", bufs=2))
        phT16_p = es.enter_context(tc.tile_pool(name="phT16", bufs=2))
        brow_p = es.enter_context(tc.tile_pool(name="brow", bufs=3))
        vraw_p = es.enter_context(tc.tile_pool(name="vraw", bufs=2))
        vsq_p = es.enter_context(tc.tile_pool(name="vsq", bufs=2))
        pcol_p = es.enter_context(tc.tile_pool(name="pcol", bufs=3))
        sb16_p = es.enter_context(tc.tile_pool(name="sb16", bufs=2))
        mt_p = es.enter_context(tc.tile_pool(name="mt", bufs=2))
        twp_p = es.enter_context(tc.tile_pool(name="twp", bufs=3))
        row_p = es.enter_context(tc.tile_pool(name="rowp", bufs=2))

        def load(j):
            phT8_t = phT8_p.tile([128, HC, LP], FP8, tag="phT8")
            nc.sync.dma_start(out=phT8_t[:], in_=d["phT8"][j])
            phT16_t = phT16_p.tile([128, HC, LP], BF16, tag="phT16")
            nc.sync.dma_start(out=phT16_t[:], in_=d["phT16"][j])
            phot_t = brow_p.tile([128, 4, 1024], BF16, tag="phot")
            nc.sync.dma_start(out=phot_t[:], in_=d["phot16"][j])
            return phT8_t, phT16_t, phot_t

        # ================= q side =========================================
        with tc.tile_pool(name="qtmp", bufs=1) as qtmp, \
             tc.tile_pool(name="qvraw", bufs=2) as qvraw_p, \
             tc.tile_pool(name="qvsq", bufs=2) as qvsq_p:
            qhT16 = qtmp.tile([128, HC, BQ * LQ], BF16, tag="qhT16")
            nc.sync.dma_start(out=qhT16[:], in_=d["qhT16"][:])
            loads = {0: load(0)}

            # colbert projection + normalize (2 column groups of 512)
            for g in range(2):
                cols = slice(g * 512, (g + 1) * 512)
                vraw_q = qvraw_p.tile([128, HC, 512], BF16, tag="qvraw")
                vsq8_q = qvsq_p.tile([128, HC, 512], FP8, tag="qvsq")
                for m in range(HC):
                    pmm = ps_mm.tile([128, 512], F32, tag="mm")
                    for t in range(HC // 2):
                        nc.tensor.matmul(
                            pmm[:],
                            wt8[:, 2 * t:2 * t + 2, m * 128:(m + 1) * 128],
                            qhT8[:, 2 * t:2 * t + 2, cols],
                            start=(t == 0), stop=(t == HC // 2 - 1),
                            perf_mode=DR)
                    if m % 2 == 0:
                        nc.scalar.activation(out=vsq8_q[:, m, :], in_=pmm[:],
                                             func=AF.Square,
                                             bias=packf[:, HC + m:HC + m + 1],
                                             scale=0.5)
                    nc.scalar.activation(out=vraw_q[:, m, :], in_=pmm[:],
                                         func=AF.Identity,
                                         bias=packf[:, HC + m:HC + m + 1],
                                         scale=0.5)
                    if m % 2 == 1:
                        nc.vector.tensor_mul(vsq8_q[:, m, :], vraw_q[:, m, :],
                                             vraw_q[:, m, :])
                pss = ps_row.tile([1, 512], F32, tag="row")
                for t in range(HC // 2):
                    nc.tensor.matmul(
                        pss[:], ones8[:, 2 * t:2 * t + 2, 0:1],
                        vsq8_q[:, 2 * t:2 * t + 2, :],
                        start=(t == 0), stop=(t == HC // 2 - 1), perf_mode=DR)
                nrow = qtmp.tile([1, 512], F32, tag=f"qnrow{g}", name=f"qnrow{g}")
                nc.scalar.activation(out=nrow[:], in_=pss[:], func=AF.Sqrt,
                                     bias=eps_sb[:], scale=1.0 / (CS * CS))
                srow = qtmp.tile([1, 512], F32R, tag=f"qsrow{g}", name=f"qsrow{g}")
                nc.vector.reciprocal(out=srow[:], in_=nrow[:])
                sB = ps_sb.tile([128, 512], F32, tag="sb")
                nc.tensor.matmul(sB[:], ones_r[:], srow[:], start=True, stop=True)
                sB16 = sb16_p.tile([128, 512], BF16, tag="sb16")
                nc.scalar.copy(out=sB16[:], in_=sB[:])
                for m in range(HC):
                    if m < 1:
                        nc.vector.tensor_mul(qcolT8[:, m, cols],
                                             vraw_q[:, m, :], sB[:])
                    else:
                        nc.gpsimd.tensor_mul(qcolT8[:, m, cols],
                                             vraw_q[:, m, :], sB16[:])

            # sparse token weights for q: PE matvec over hidden chunks,
            # relu row, DRAM bounce to per-token columns
            twq_row = qtmp.tile([1, BQ * LQ], F32R, tag="twq_row")
            for g in range(2):
                ptw = ps_row.tile([1, 512], F32, tag="row")
                for k in range(HC):
                    nc.tensor.matmul(ptw[:], pack16[:, k, 16:17],
                                     qhT16[:, k, g * 512:(g + 1) * 512],
                                     start=(k == 0), stop=(k == HC - 1))
                nc.scalar.activation(out=twq_row[:, g * 512:(g + 1) * 512],
                                     in_=ptw[:], func=AF.Relu, bias=packf[0:1, 32:33],
                                     scale=1.0)
            nc.gpsimd.dma_start(out=d_twq[:], in_=twq_row[:])
            for i in range(BQ):
                nc.gpsimd.dma_start(
                    out=qwB[i][:],
                    in_=bass.AP(tensor=d_twq.tensor, offset=i * LQ,
                                ap=[[0, BQ], [1, LQ]]))

            # qlen -> 1/(qlen*TEMP*CS*CS) row for the final colbert scale
            pql = ps_row.tile([1, BQ], F32, tag="row")
            nc.tensor.matmul(pql[:], ones_cq[0:127, :], qmT[0:127, :],
                             start=True, stop=True)
            nc.vector.tensor_scalar(out=qiv_row[:], in0=pql[:],
                                    scalar1=TEMP * CS * CS,
                                    scalar2=None, op0=AX.mult)
            nc.vector.reciprocal(out=qiv_row[:], in_=qiv_row[:])

            # dense scores (emitted here so they overlap the passage loop;
            # output DMA happens in the finals)
            dn = qtmp.tile([BQ, BP], F32, tag="dn")
            qsq = qtmp.tile([128, HC, BQ], BF16, tag="qsq")
            psq = qtmp.tile([128, HC, BP], BF16, tag="psq")
            for k in range(HC):
                nc.scalar.activation(out=qsq[:, k, :], in_=pack16[:, k, 0:BQ],
                                     func=AF.Square)
                nc.scalar.activation(out=psq[:, k, :], in_=pack16[:, k, BQ:BQ + BP],
                                     func=AF.Square)
            pd = ps_row.tile([BQ, BP], F32, tag="row")
            for k in range(HC):
                nc.tensor.matmul(pd[:], pack16[:, k, 0:BQ], pack16[:, k, BQ:BQ + BP],
                                 start=(k == 0), stop=(k == HC - 1))
            pdsb = qtmp.tile([BQ, BP], F32, tag="pdsb")
            nc.scalar.copy(out=pdsb[:], in_=pd[:])
            pqn = ps_row.tile([1, BQ], F32, tag="row")
            for k in range(HC):
                nc.tensor.matmul(pqn[:], ones_c16[:], qsq[:, k, :],
                                 start=(k == 0), stop=(k == HC - 1))
            rq_row = qtmp.tile([1, BQ], F32, tag="rq_row")
            nc.scalar.activation(out=rq_row[:], in_=pqn[:], func=AF.Sqrt,
                                 bias=eps_sb[:])
            nc.vector.reciprocal(out=rq_row[:], in_=rq_row[:])
            ppn = ps_row.tile([1, BP], F32, tag="row")
            for k in range(HC):
                nc.tensor.matmul(ppn[:], ones_c16[:], psq[:, k, :],
                                 start=(k == 0), stop=(k == HC - 1))
            rp_row = qtmp.tile([1, BP], F32R, tag="rp_row")
            nc.scalar.activation(out=rp_row[:], in_=ppn[:], func=AF.Sqrt,
                                 bias=eps_sb[:])
            nc.vector.reciprocal(out=rp_row[:], in_=rp_row[:])
            nc.gpsimd.dma_start(out=d_rq[:], in_=rq_row[:])
            rq_col = qtmp.tile([BQ, 1], F32, tag="rq_col")
            nc.gpsimd.dma_start(
                out=rq_col[:],
                in_=bass.AP(tensor=d_rq.tensor, offset=0, ap=[[1, BQ], [0, 1]]))
            prpb = ps_row.tile([BQ, BP], F32, tag="row")
            nc.tensor.matmul(prpb[:], ones_r[:, 0:BQ], rp_row[:],
                             start=True, stop=True)
            dmul = qtmp.tile([BQ, BP], F32, tag="dmul")
            nc.vector.tensor_mul(dmul[:], pdsb[:], prpb[:])
            nc.vector.tensor_scalar(out=dn[:], in0=dmul[:], scalar1=rq_col[:],
                                    scalar2=1.0 / TEMP, op0=AX.mult, op1=AX.mult)
            nc.sync.dma_start(out=o["dense"][:], in_=dn[:])

        # ================= passage loop ===================================
        # steady-state PE order per iteration j:
        #   proj(j) x32, psc(j-1) q0..6, norm(j) x4, psc(j-1) q7, sB(j)
        state = {}

        def proj_tw(j):
            phT8_t, phT16_t, phot_t = loads.pop(j)
            vraw = vraw_p.tile([128, HC, LP], BF16, tag="vraw")
            vsq8 = vsq_p.tile([128, HC, LP], FP8, tag="vsq")
            for m in range(HC):
                pmm = ps_mm.tile([128, LP], F32, tag="mm")
                for t in range(HC // 2):
                    nc.tensor.matmul(
                        pmm[:],
                        wt8[:, 2 * t:2 * t + 2, m * 128:(m + 1) * 128],
                        phT8_t[:, 2 * t:2 * t + 2, :],
                        start=(t == 0), stop=(t == HC // 2 - 1), perf_mode=DR)
                # vraw holds v/2; even-m squares on ACT (straight off PSUM),
                # odd-m on DVE from the halved vraw (fp8-safe)
                if m % 2 == 0:
                    nc.scalar.activation(out=vsq8[:, m, :], in_=pmm[:],
                                         func=AF.Square,
                                         bias=packf[:, HC + m:HC + m + 1],
                                         scale=0.5)
                nc.scalar.activation(out=vraw[:, m, :], in_=pmm[:],
                                     func=AF.Identity,
                                     bias=packf[:, HC + m:HC + m + 1],
                                     scale=0.5)
                if m % 2 == 1:
                    nc.vector.tensor_mul(vsq8[:, m, :], vraw[:, m, :],
                                         vraw[:, m, :])
            # sparse token weights: PE matvec, relu row, bounce + broadcast
            ptw = ps_row.tile([1, LP], F32, tag="row")
            for k in range(HC):
                nc.tensor.matmul(ptw[:], pack16[:, k, 16:17], phT16_t[:, k, :],
                                 start=(k == 0), stop=(k == HC - 1))
            twp_row = twp_p.tile([1, LP], F32R, tag="twp_row")
            nc.scalar.activation(out=twp_row[:], in_=ptw[:], func=AF.Relu,
                                 bias=sb_sb[:], scale=1.0)
            nc.gpsimd.dma_start(out=d_twp[j:j + 1, :], in_=twp_row[:])
            twpB = brow_p.tile([128, LP], F32R, tag="twpB")
            nc.gpsimd.dma_start(out=twpB[:], in_=_bcast_rows(d_twp[j:j + 1, :]))
            state[j] = [vraw, vsq8, twpB, pidB, None]

        def norm(j):
            vsq8 = state[j][1]
            pss = ps_row.tile([1, LP], F32, tag="row")
            for t in range(HC // 2):
                nc.tensor.matmul(
                    pss[:], ones8[:, 2 * t:2 * t + 2, 0:1],
                    vsq8[:, 2 * t:2 * t + 2, :],
                    start=(t == 0), stop=(t == HC // 2 - 1), perf_mode=DR)
            return pss

        def scale(j, pss):
            vraw = state[j][0]
            nrow = row_p.tile([1, LP], F32, tag="nrow")
            nc.scalar.activation(out=nrow[:], in_=pss[:], func=AF.Sqrt,
                                 bias=eps_sb[:], scale=1.0 / (CS * CS))
            srow = row_p.tile([1, LP], F32R, tag="srow")
            nc.vector.reciprocal(out=srow[:], in_=nrow[:])
            sB = ps_sb.tile([128, LP], F32, tag="sb")
            nc.tensor.matmul(sB[:], ones_r[:], srow[:], start=True, stop=True)
            sB16 = sb16_p.tile([128, LP], BF16, tag="sb16")
            nc.scalar.copy(out=sB16[:], in_=sB[:])
            pcolT8 = pcol_p.tile([128, HC, LP], FP8, tag="pct8")
            for m in range(HC):
                if m < 1:
                    nc.vector.tensor_mul(pcolT8[:, m, :], vraw[:, m, :], sB[:])
                else:
                    nc.gpsimd.tensor_mul(pcolT8[:, m, :], vraw[:, m, :],
                                         sB16[:])
            state[j][4] = pcolT8

        def sparse_s(j):
            # s_j[a] = sum_l phot[l, a] * twp[l]  (exact 0/1 one-hot GEMM)
            _, _, twp4c, phot_t, _ = state[j]
            ps_s = ps_row.tile([128, HC], F32, tag="row")
            for ac in range(HC):
                for lc in range(4):
                    nc.tensor.matmul(
                        ps_s[:, ac:ac + 1],
                        phot_t[:, lc, ac * 128:(ac + 1) * 128],
                        twp4c[:, lc:lc + 1],
                        start=(lc == 0), stop=(lc == 3))
            nc.scalar.copy(out=S_sb[:, :, j], in_=ps_s[:])

        def scores(j, qs):
            pcolT8 = state[j][4]
            for i in qs:
                psc = ps_mm.tile([127, LP], F32, tag="mm")
                for t in range(HC // 2):
                    nc.tensor.matmul(
                        psc[:],
                        qcolT8[:, 2 * t:2 * t + 2, i * 128 + 1:(i + 1) * 128],
                        pcolT8[:, 2 * t:2 * t + 2, :],
                        start=(t == 0), stop=(t == HC // 2 - 1), perf_mode=DR)
                nc.vector.reduce_max(
                    out=rmax_all[0:127, i * BP + j:i * BP + j + 1],
                    in_=psc[:, 1:LP], axis=X)

        for j in range(BP):
            if j + 1 < BP:
                loads[j + 1] = load(j + 1)
            proj_tw(j)
            if j > 0:
                sparse_s(j - 1)
                scores(j - 1, range(0, 7))
            pss = norm(j)
            if j > 0:
                scores(j - 1, range(7, 8))
                del state[j - 1]
            scale(j, pss)
        sparse_s(BP - 1)
        scores(BP - 1, range(BQ))
        del state[BP - 1]

        # ================= finals =========================================
        with tc.tile_pool(name="fin", bufs=1) as fin:
            # colbert: one 64-col partition-sum matmul, then scale by the
            # block-repeated 1/(qlen*TEMP*CS*CS) row
            pcb = ps_row.tile([1, BQ * BP], F32, tag="row")
            nc.tensor.matmul(pcb[:], ones_cq[0:127, :], rmax_all[0:127, :],
                             start=True, stop=True)
            cst = fin.tile([1, BQ * BP], F32, tag="cst")
            for i in range(BQ):
                nc.vector.tensor_scalar(
                    out=cst[:, i * BP:(i + 1) * BP],
                    in0=pcb[:, i * BP:(i + 1) * BP],
                    scalar1=qiv_row[0:1, i:i + 1], scalar2=None, op0=AX.mult)
            nc.sync.dma_start(
                out=bass.AP(tensor=o["colbert"].ap().tensor, offset=0,
                            ap=[[0, 1], [1, BQ * BP]]),
                in_=cst[:])

            # sparse stage 2: ssumT_i[j, p] = sum_a S[a, j] * qhot_i[a, p],
            # then contract q token weights along the free axis and scale
            spT = fin.tile([BQ, BQ], F32, tag="spT")
            scr8 = fin.tile([BQ, 128], F32, tag="scr8")
            for i in range(BQ):
                psT = ps_mm.tile([BP, 128], F32, tag="mm")
                for ac in range(HC):
                    nc.tensor.matmul(psT[:], S_sb[:, ac, :],
                                     qhot16[:, ac, i, :],
                                     start=(ac == 0), stop=(ac == HC - 1))
                nc.vector.scalar_tensor_tensor(
                    out=scr8[:], in0=psT[:], scalar=1.0, in1=qwB[i][:],
                    op0=AX.mult, op1=AX.mult,
                    accum_out=spT[:, i:i + 1])
            spTs = fin.tile([BQ, BQ], F32, tag="spTs")
            nc.vector.tensor_scalar(out=spTs[:], in0=spT[:],
                                    scalar1=1.0 / TEMP, scalar2=None,
                                    op0=AX.mult)
            nc.gpsimd.dma_start(
                out=bass.AP(tensor=o["sparse"].ap().tensor, offset=0,
                            ap=[[1, BP], [BP, BQ]]),
                in_=spTs[:])
